# revision 16
# baseline (speedup 1.0000x reference)
"""Trainium2 Bass kernel for nn_CNN_Comp_29240137351522 (dense_cnn).

Math:  y = |IFFT_N( FFT_N(x)^2 * C )[255:2303]|,  C = FFT_N(w0)^2 * FFT_N(wl) / N
with N = 2560 >= 2559 so the chained full convolutions (x*w0, autoconv, *wl)
are exact linear convolutions.

Device decomposition (per core, data-parallel over batch):
  N = N2*N1, N1=128, N2=20;  time n = n2*128+n1,  freq k = k1*20+k2
  F1 (contract n2, PE, block-diag over n1 i-blocks of 4, twiddle folded)
  F3 (contract n1, PE, shared W128 DFT)          -> X[k1, (k2,b)]
  square (ACT/DVE fused into F3 eviction)         -> Zr = Xr^2-Xi^2, P = Xr*Xi
  I1 (contract k1, PE, per-k2 weights G = C-row-scaled inverse DFT; the
      factor 2 of Zi=2P folded into G variants), bf16
  I2 (contract k2, PE, block-diag over n1 i-blocks of {6,6,4}, twiddle folded,
      output n2 in [1,18)), bf16
  |.| fused into I2 eviction; raw tiles stored to DRAM, unscrambled on host.

Host does data movement only: batch shard, column permutation of x (so PE
transposes produce the (i,n2)-partition layout directly), and the inverse
row->output-column unscramble of the raw result.
"""

import numpy as np
import ml_dtypes

import concourse.bass as bass
import concourse.bacc as bacc
import concourse.mybir as mybir
from concourse.tile import TileContext
from concourse.bass_utils import run_bass_kernel_spmd

# ---------------- static problem config ----------------
B, NX = 4096, 1024
K0, KL = 129, 257
N = 2560
N1, N2 = 128, 20
NCORES = 8
BCORE = B // NCORES          # 512
CHUNK = 256
NCHUNKS = BCORE // CHUNK     # 2
N2OUT = 17                   # n2 in [1,18)
CROP0 = 255
CLASS_NUM = 2048
IBLK_I2 = (6, 6, 4)
JOFS_I2 = (0, 6, 12)
YRAW_ROWS = 8 * sum(IBLK_I2) * N2OUT  # 2176

f32 = mybir.dt.float32
f32r = mybir.dt.float32r
bf16 = mybir.dt.bfloat16
AO = mybir.AluOpType
AF = mybir.ActivationFunctionType


def _w(num, den):
    return np.exp(-2j * np.pi * np.asarray(num, np.float64) / den)


# ---------------- host-side constant arrays ----------------
def _build_consts():
    c = {}
    n1g = np.arange(N1)
    k1g = np.arange(N1)
    k2g = np.arange(N2)
    n2g8 = np.arange(8)

    # F1 lhsT: [128, 640]; block (g,jj) at partitions [32jj,32jj+32), cols [80g,80g+80)
    # rows (il in 4)*8 + n2, cols il*20 + k2; value W20[n2,k2] * W2560^{n1 k2}, n1=16g+4jj+il
    f1 = np.zeros((128, 640), np.complex128)
    for g in range(8):
        for jj in range(4):
            for il in range(4):
                n1 = 16 * g + 4 * jj + il
                blk = _w(np.outer(n2g8, k2g), N2) * _w(n1 * k2g, N)[None, :]
                f1[32 * jj + il * 8 : 32 * jj + il * 8 + 8, 80 * g + il * 20 : 80 * g + (il + 1) * 20] = blk
    c["cf1r"] = f1.real.astype(np.float32)
    c["cf1i"] = f1.imag.astype(np.float32)
    c["cf1n"] = (-f1.imag).astype(np.float32)

    # F3 lhsT (shared): W128[n1,k1]
    w3 = _w(np.outer(n1g, k1g), N1)
    c["cw3r"] = w3.real.astype(np.float32)
    c["cw3i"] = w3.imag.astype(np.float32)
    c["cw3n"] = (-w3.imag).astype(np.float32)

    # I1 base: W128i[k1,n1] (fp32, G built on device)
    wi = _w(-np.outer(k1g, n1g), N1)
    c["cwir"] = wi.real.astype(np.float32)
    c["cwii"] = wi.imag.astype(np.float32)

    # I2 lhsT: [120, 2176]; per (g,j) cols [off,off+M_j); block-diag il:
    # rows il*20+k2, cols il*17+(n2-1); value W20^{-k2 n2} * W2560^{-n1 k2}
    n2out = np.arange(1, 18)
    i2 = np.zeros((120, 2176), np.complex128)
    off = 0
    for g in range(8):
        for j, cnt in enumerate(IBLK_I2):
            for il in range(cnt):
                n1 = 16 * g + JOFS_I2[j] + il
                blk = _w(-np.outer(k2g, n2out), N2) * _w(-n1 * k2g, N)[:, None]
                i2[il * 20 : (il + 1) * 20, off + il * 17 : off + (il + 1) * 17] = blk
            off += cnt * N2OUT
    c["ci2r"] = i2.real.astype(ml_dtypes.bfloat16)
    c["ci2i"] = i2.imag.astype(ml_dtypes.bfloat16)
    c["ci2n"] = (-i2.imag).astype(ml_dtypes.bfloat16)

    # weight-DFT rhs constants
    nh = np.arange(128)
    t129 = _w(np.outer(nh, k2g), N)
    c["ct1r"] = t129.real.astype(np.float32)
    c["ct1i"] = t129.imag.astype(np.float32)
    t257b = _w(np.outer(nh, k2g), N) * _w(k2g, 20)[None, :]
    c["ct2r"] = t257b.real.astype(np.float32)
    c["ct2i"] = t257b.imag.astype(np.float32)
    t129e = _w(k2g, 20)
    c["te1r"] = t129e.real.astype(np.float32).reshape(1, N2)
    c["te1i"] = t129e.imag.astype(np.float32).reshape(1, N2)
    t257e = _w(k2g, 10)
    c["te2r"] = t257e.real.astype(np.float32).reshape(1, N2)
    c["te2i"] = t257e.imag.astype(np.float32).reshape(1, N2)

    c["ones1"] = np.ones((1, 128), np.float32)
    c["ident"] = np.eye(128, dtype=np.float32)
    return c


CONSTS = _build_consts()


def host_x_perm():
    """perm[g*128 + i*8 + n2] = n2*128 + 16g + i"""
    perm = np.empty(NX, np.int64)
    for g in range(8):
        for i in range(16):
            for n2 in range(8):
                perm[g * 128 + i * 8 + n2] = n2 * 128 + 16 * g + i
    return perm


def yraw_maps():
    """row r of yraw -> output column (n-255), valid mask."""
    rows = []
    for g in range(8):
        for j, cnt in enumerate(IBLK_I2):
            for il in range(cnt):
                n1 = 16 * g + JOFS_I2[j] + il
                for q in range(N2OUT):
                    rows.append((q + 1) * 128 + n1)
    narr = np.array(rows)
    valid = (narr >= CROP0) & (narr < CROP0 + CLASS_NUM)
    return narr, valid


XPERM = host_x_perm()
YN, YVALID = yraw_maps()


# ---------------- bass kernel builder ----------------
def build_nc():
    nc = bacc.Bacc("TRN2", target_bir_lowering=False, debug=False, num_devices=NCORES)

    # DRAM tensors
    d = {}
    d["xp_r"] = nc.dram_tensor("xp_r", [BCORE, NX], f32, kind="ExternalInput")
    d["xp_i"] = nc.dram_tensor("xp_i", [BCORE, NX], f32, kind="ExternalInput")
    for nm, shape in [("w0r", [K0]), ("w0i", [K0]), ("wlr", [KL]), ("wli", [KL])]:
        d[nm] = nc.dram_tensor(nm, shape, f32, kind="ExternalInput")
    cdt = {"cf1r": f32r, "cf1i": f32r, "cf1n": f32r,
           "cw3r": f32r, "cw3i": f32r, "cw3n": f32r,
           "ci2r": bf16, "ci2i": bf16, "ci2n": bf16,
           "ones1": f32r}
    for nm, arr in CONSTS.items():
        d[nm] = nc.dram_tensor(nm, list(arr.shape), cdt.get(nm, f32), kind="ExternalInput")
    yraw = nc.dram_tensor("yraw", [YRAW_ROWS, BCORE], f32, kind="ExternalOutput")

    with TileContext(nc) as tc:
        with (
            tc.tile_pool(name="cp", bufs=1) as cp,         # consts + persistent
            tc.tile_pool(name="bp", bufs=1) as bp,         # big per-chunk tiles
            tc.tile_pool(name="sp", bufs=6) as sp,         # small rotating tiles
            tc.tile_pool(name="tp", bufs=3) as tp,         # f32 tmp tiles
            tc.tile_pool(name="psa", bufs=2, space="PSUM") as psa,   # 4 tags x 2 bufs = 8 banks
        ):
            # ---- load constants ----
            ct = {}
            big_consts = {"ci2r", "ci2i", "ci2n", "cwir", "cwii"}
            for nm, arr in CONSTS.items():
                t = cp.tile(list(arr.shape), cdt.get(nm, f32), tag=nm)
                eng = nc.gpsimd if nm in big_consts else nc.sync
                eng.dma_start(out=t[:], in_=d[nm][:, :] if arr.ndim == 2 else d[nm][:])
                ct[nm] = t

            # ---- load w0/wl pieces as [128,1] / [1,1] columns ----
            wc = {}
            for nm, src, lo, hi in [
                ("w0r_c", "w0r", 0, 128), ("w0i_c", "w0i", 0, 128),
                ("wlr_c1", "wlr", 0, 128), ("wli_c1", "wli", 0, 128),
                ("wlr_c2", "wlr", 128, 256), ("wli_c2", "wli", 128, 256),
            ]:
                t = cp.tile([128, 1], f32, tag=nm)
                nc.sync.dma_start(out=t[:], in_=d[src][lo:hi])
                wc[nm] = t
            for nm, src, pos in [("w0r_e", "w0r", 128), ("w0i_e", "w0i", 128),
                                 ("wlr_e", "wlr", 256), ("wli_e", "wli", 256)]:
                t = cp.tile([1, 1], f32, tag=nm)
                nc.sync.dma_start(out=t[:], in_=d[src][pos:pos + 1])
                wc[nm] = t

            # ---- weight DFT: W0, WL [128, 20] ----
            def build_rhs(tr, ti, cr_, ci_, out_r, out_i):
                # out_r = tr*cr - ti*ci ; out_i = ti*cr + tr*ci   (complex (tr+i ti)*(cr+i ci))
                tmp = tp.tile([tr.shape[0], N2], f32, tag="wtmp")
                nc.vector.tensor_scalar(tmp[:], ti[:], ci_[:], None, AO.mult)
                nc.vector.scalar_tensor_tensor(out_r[:], tr[:], cr_[:], tmp[:], AO.mult, AO.subtract)
                tmp2 = tp.tile([tr.shape[0], N2], f32, tag="wtmp2")
                nc.vector.tensor_scalar(tmp2[:], tr[:], ci_[:], None, AO.mult)
                nc.vector.scalar_tensor_tensor(out_i[:], ti[:], cr_[:], tmp2[:], AO.mult, AO.add)

            def weight_dft(chunks, tail, out_r, out_i):
                """chunks: list of (t_r_tile, t_i_tile, colr, coli); tail: (te_r, te_i, er, ei)."""
                ps_r = psa.tile([128, N2], f32, tag="pAr")
                ps_i = psa.tile([128, N2], f32, tag="pAi")
                rhs = []
                for (t_r, t_i, colr, coli) in chunks:
                    rr = sp.tile([128, N2], f32r, tag="wrhs_r")
                    ri = sp.tile([128, N2], f32r, tag="wrhs_i")
                    build_rhs(t_r, t_i, colr, coli, rr, ri)
                    rhs.append((rr, ri))
                te_r, te_i, er, ei = tail
                tr = sp.tile([1, N2], f32r, tag="wtail_r")
                ti_ = sp.tile([1, N2], f32r, tag="wtail_i")
                tmp = tp.tile([1, N2], f32, tag="wtmp3")
                nc.vector.tensor_scalar(tmp[:], te_i[:], ei[:], None, AO.mult)
                nc.vector.scalar_tensor_tensor(tr[:], te_r[:], er[:], tmp[:], AO.mult, AO.subtract)
                tmp2 = tp.tile([1, N2], f32, tag="wtmp4")
                nc.vector.tensor_scalar(tmp2[:], te_r[:], ei[:], None, AO.mult)
                nc.vector.scalar_tensor_tensor(ti_[:], te_i[:], er[:], tmp2[:], AO.mult, AO.add)
                # psum groups
                first = True
                for (rr, ri) in rhs:
                    nc.tensor.matmul(ps_r[:], ct["cw3r"][:], rr[:], start=first, stop=False)
                    nc.tensor.matmul(ps_r[:], ct["cw3n"][:], ri[:], start=False, stop=False)
                    first = False
                nc.tensor.matmul(ps_r[:], ct["ones1"][:1, :], tr[:], start=False, stop=True)
                first = True
                for (rr, ri) in rhs:
                    nc.tensor.matmul(ps_i[:], ct["cw3i"][:], rr[:], start=first, stop=False)
                    nc.tensor.matmul(ps_i[:], ct["cw3r"][:], ri[:], start=False, stop=False)
                    first = False
                nc.tensor.matmul(ps_i[:], ct["ones1"][:1, :], ti_[:], start=False, stop=True)
                nc.vector.tensor_copy(out_r[:], ps_r[:])
                nc.vector.tensor_copy(out_i[:], ps_i[:])

            W0r = cp.tile([128, N2], f32, tag="W0r")
            W0i = cp.tile([128, N2], f32, tag="W0i")
            weight_dft(
                [(ct["ct1r"], ct["ct1i"], wc["w0r_c"], wc["w0i_c"])],
                (ct["te1r"], ct["te1i"], wc["w0r_e"], wc["w0i_e"]),
                W0r, W0i,
            )
            WLr = cp.tile([128, N2], f32, tag="WLr")
            WLi = cp.tile([128, N2], f32, tag="WLi")
            weight_dft(
                [(ct["ct1r"], ct["ct1i"], wc["wlr_c1"], wc["wli_c1"]),
                 (ct["ct2r"], ct["ct2i"], wc["wlr_c2"], wc["wli_c2"])],
                (ct["te2r"], ct["te2i"], wc["wlr_e"], wc["wli_e"]),
                WLr, WLi,
            )

            # ---- C = W0^2 * WL / N  [128, 20] ----
            Cr = cp.tile([128, N2], f32, tag="Cr")
            Ci = cp.tile([128, N2], f32, tag="Ci")
            ta = tp.tile([128, N2], f32, tag="ca")
            tb = tp.tile([128, N2], f32, tag="cb")
            tm1 = tp.tile([128, N2], f32, tag="cm1")
            tm2 = tp.tile([128, N2], f32, tag="cm2")
            nc.vector.tensor_mul(tm1[:], W0r[:], W0r[:])
            nc.vector.tensor_mul(tm2[:], W0i[:], W0i[:])
            nc.vector.tensor_sub(ta[:], tm1[:], tm2[:])          # a = W0r^2 - W0i^2
            nc.vector.tensor_mul(tm1[:], W0r[:], W0i[:])
            nc.vector.tensor_add(tb[:], tm1[:], tm1[:])          # b = 2 W0r W0i
            nc.vector.tensor_mul(tm1[:], ta[:], WLr[:])
            nc.vector.tensor_mul(tm2[:], tb[:], WLi[:])
            nc.vector.tensor_sub(tm1[:], tm1[:], tm2[:])
            nc.scalar.mul(Cr[:], tm1[:], 1.0 / N)
            nc.vector.tensor_mul(tm1[:], ta[:], WLi[:])
            nc.vector.tensor_mul(tm2[:], tb[:], WLr[:])
            nc.vector.tensor_add(tm1[:], tm1[:], tm2[:])
            nc.scalar.mul(Ci[:], tm1[:], 1.0 / N)

            # ---- G variants (bf16): G_k2 = C[:,k2] row-scaled W128i ----
            Gr = cp.tile([128, N2 * 128], bf16, tag="Gr")
            Gi = cp.tile([128, N2 * 128], bf16, tag="Gi")
            Gn2 = cp.tile([128, N2 * 128], bf16, tag="Gn2")  # -2*Gi
            Gr2 = cp.tile([128, N2 * 128], bf16, tag="Gr2")  # 2*Gr
            for k2 in range(N2):
                cr_ = Cr[:, k2 : k2 + 1]
                ci_ = Ci[:, k2 : k2 + 1]
                sl = slice(k2 * 128, (k2 + 1) * 128)
                gt = tp.tile([128, 128], f32, tag="gtmp")
                nc.vector.tensor_scalar(gt[:], ct["cwii"][:], ci_, None, AO.mult)
                nc.vector.scalar_tensor_tensor(Gr[:, sl], ct["cwir"][:], cr_, gt[:], AO.mult, AO.subtract)
                gt2 = tp.tile([128, 128], f32, tag="gtmp2")
                nc.vector.tensor_scalar(gt2[:], ct["cwir"][:], ci_, None, AO.mult)
                nc.vector.scalar_tensor_tensor(Gi[:, sl], ct["cwii"][:], cr_, gt2[:], AO.mult, AO.add)
                nc.scalar.mul(Gn2[:, sl], Gi[:, sl], -2.0)
                nc.scalar.mul(Gr2[:, sl], Gr[:, sl], 2.0)

            # ---- per-chunk pipeline ----
            i2_offs = []
            off = 0
            for g in range(8):
                for j, cnt in enumerate(IBLK_I2):
                    i2_offs.append((g, j, cnt, off))
                    off += cnt * N2OUT

            for c in range(NCHUNKS):
                # T-in: load + transpose
                xn_r = bp.tile([128, 2048], f32, tag="big1")
                xn_i = bp.tile([128, 2048], f32, tag="big2")
                for h in range(2):
                    rows = slice(c * CHUNK + h * 128, c * CHUNK + (h + 1) * 128)
                    nc.sync.dma_start(out=xn_r[:, h * 1024 : (h + 1) * 1024], in_=d["xp_r"][rows, :])
                    nc.sync.dma_start(out=xn_i[:, h * 1024 : (h + 1) * 1024], in_=d["xp_i"][rows, :])
                xt_r = bp.tile([128, 2048], f32r, tag="big3")
                xt_i = bp.tile([128, 2048], f32r, tag="big4")
                for plane, xn, xt in [(0, xn_r, xt_r), (1, xn_i, xt_i)]:
                    for h in range(2):
                        for g in range(8):
                            tps = psa.tile([128, 512], f32, tag="pBr")
                            nc.tensor.transpose(
                                tps[:128, :128],
                                xn[:, h * 1024 + g * 128 : h * 1024 + (g + 1) * 128],
                                ct["ident"][:],
                            )
                            nc.scalar.activation(
                                xt[:, g * 256 + h * 128 : g * 256 + (h + 1) * 128],
                                tps[:128, :128], AF.Copy,
                            )

                # F1 + pivot-C into plane-interleaved Abig [n1, k2*512 + plane*256 + b]
                Abig = bp.tile([128, 10240], f32r, tag="Abig")
                for g in range(8):
                    for jj in range(4):
                        pw = slice(32 * jj, 32 * jj + 32)
                        cwd = slice(80 * g, 80 * (g + 1))
                        rr = xt_r[pw, g * 256 : (g + 1) * 256]
                        ri = xt_i[pw, g * 256 : (g + 1) * 256]
                        lr = ct["cf1r"][pw, cwd]
                        li = ct["cf1i"][pw, cwd]
                        ln = ct["cf1n"][pw, cwd]
                        tpos = (32 * jj, 0)
                        pr = psa.tile([80, 256], f32, tag="pAr")
                        pi = psa.tile([80, 256], f32, tag="pAi")
                        nc.tensor.matmul(pr[:], lr, rr, start=True, stop=False, tile_position=tpos)
                        nc.tensor.matmul(pr[:], ln, ri, start=False, stop=True, tile_position=tpos)
                        nc.tensor.matmul(pi[:], li, rr, start=True, stop=False, tile_position=tpos)
                        nc.tensor.matmul(pi[:], lr, ri, start=False, stop=True, tile_position=tpos)
                        ag = sp.tile([80, 512], f32r, tag="ag")
                        nc.scalar.activation(ag[:, 0:256], pr[:], AF.Copy)
                        nc.vector.tensor_copy(ag[:, 256:512], pi[:])
                        # pivot: [(il,k2), (plane,b)] -> Abig[n1, k2*512+plane*256+b]
                        nc.sync.dma_start(
                            out=bass.AP(Abig.tensor, Abig[:].offset + (16 * g + 4 * jj) * 10240,
                                        [[10240, 4], [1, 10240]]),
                            in_=ag[:],
                        )

                # F3 + fused square eviction
                Zr = bp.tile([128, 5120], bf16, tag="Zr")
                Pt = bp.tile([128, 5120], bf16, tag="Pt")
                for k2 in range(N2):
                    asl_r = slice(k2 * 512, k2 * 512 + 256)
                    asl_i = slice(k2 * 512 + 256, k2 * 512 + 512)
                    zsl = slice(k2 * 256, (k2 + 1) * 256)
                    pr = psa.tile([128, 256], f32, tag="pBr")
                    pi = psa.tile([128, 256], f32, tag="pBi")
                    nc.tensor.matmul(pr[:], ct["cw3r"][:], Abig[:, asl_r], start=True, stop=False)
                    nc.tensor.matmul(pr[:], ct["cw3n"][:], Abig[:, asl_i], start=False, stop=True)
                    nc.tensor.matmul(pi[:], ct["cw3i"][:], Abig[:, asl_r], start=True, stop=False)
                    nc.tensor.matmul(pi[:], ct["cw3r"][:], Abig[:, asl_i], start=False, stop=True)
                    m1 = tp.tile([128, 256], f32, tag="sq1")
                    m2 = tp.tile([128, 256], f32, tag="sq2")
                    xi_s = tp.tile([128, 256], f32, tag="xis")
                    nc.vector.tensor_copy(xi_s[:], pi[:])
                    nc.scalar.activation(m1[:], pr[:], AF.Square)
                    nc.scalar.activation(m2[:], pi[:], AF.Square)
                    nc.vector.tensor_sub(Zr[:, zsl], m1[:], m2[:])
                    nc.vector.tensor_mul(Pt[:, zsl], pr[:], xi_s[:])

                # I1 (bf16); evict into plane-interleaved Ubig
                Ubig = bp.tile([128, 10240], bf16, tag="big1")
                for k2 in range(N2):
                    zsl = slice(k2 * 256, (k2 + 1) * 256)
                    gsl = slice(k2 * 128, (k2 + 1) * 128)
                    pr = psa.tile([128, 256], f32, tag="pAr")
                    pi = psa.tile([128, 256], f32, tag="pAi")
                    nc.tensor.matmul(pr[:], Gr[:, gsl], Zr[:, zsl], start=True, stop=False)
                    nc.tensor.matmul(pr[:], Gn2[:, gsl], Pt[:, zsl], start=False, stop=True)
                    nc.tensor.matmul(pi[:], Gi[:, gsl], Zr[:, zsl], start=True, stop=False)
                    nc.tensor.matmul(pi[:], Gr2[:, gsl], Pt[:, zsl], start=False, stop=True)
                    nc.scalar.activation(Ubig[:, k2 * 512 : k2 * 512 + 256], pr[:], AF.Copy)
                    nc.vector.tensor_copy(Ubig[:, k2 * 512 + 256 : (k2 + 1) * 512], pi[:])

                # pivot-D: one DMA per (g,j) into interleaved u2 [(il,k2), idx*512+plane*256+b]
                u2 = bp.tile([120, 24 * 512], bf16, tag="big2")
                for idx, (g, j, cnt, off) in enumerate(i2_offs):
                    n1_0 = 16 * g + JOFS_I2[j]
                    nc.sync.dma_start(
                        out=bass.AP(u2.tensor, u2[:].offset + idx * 512,
                                    [[24 * 512, cnt * 20], [1, 512]]),
                        in_=bass.AP(Ubig.tensor, Ubig[:].offset + n1_0 * 10240,
                                    [[10240, cnt], [1, 10240]]),
                    )

                # I2 (bf16) + fused abs + store
                for idx, (g, j, cnt, off) in enumerate(i2_offs):
                    Kj, Mj = cnt * 20, cnt * N2OUT
                    csl = slice(off, off + Mj)
                    usl_r = slice(idx * 512, idx * 512 + 256)
                    usl_i = slice(idx * 512 + 256, (idx + 1) * 512)
                    pr = psa.tile([102, 256], f32, tag="pBr")
                    pi = psa.tile([102, 256], f32, tag="pBi")
                    nc.tensor.matmul(pr[:Mj, :], ct["ci2r"][:Kj, csl], u2[:Kj, usl_r], start=True, stop=False)
                    nc.tensor.matmul(pr[:Mj, :], ct["ci2n"][:Kj, csl], u2[:Kj, usl_i], start=False, stop=True)
                    nc.tensor.matmul(pi[:Mj, :], ct["ci2i"][:Kj, csl], u2[:Kj, usl_r], start=True, stop=False)
                    nc.tensor.matmul(pi[:Mj, :], ct["ci2r"][:Kj, csl], u2[:Kj, usl_i], start=False, stop=True)
                    s1 = tp.tile([102, 256], f32, tag="ab1")
                    s2 = tp.tile([102, 256], f32, tag="ab2")
                    nc.scalar.activation(s1[:Mj, :], pr[:Mj, :], AF.Square)
                    nc.scalar.activation(s2[:Mj, :], pi[:Mj, :], AF.Square)
                    nc.vector.tensor_add(s1[:Mj, :], s1[:Mj, :], s2[:Mj, :])
                    ya = sp.tile([102, 256], f32, tag="yab")
                    nc.scalar.activation(ya[:Mj, :], s1[:Mj, :], AF.Sqrt)
                    nc.gpsimd.dma_start(
                        out=yraw[off : off + Mj, c * CHUNK : (c + 1) * CHUNK],
                        in_=ya[:Mj, :],
                    )

    nc.compile()
    return nc


_NC_CACHE = None
_LAST_IN_MAPS = None


def kernel(**inputs):
    global _NC_CACHE
    x_real = np.ascontiguousarray(inputs["x_real"], dtype=np.float32)
    x_imag = np.ascontiguousarray(inputs["x_imag"], dtype=np.float32)
    w0_real = np.ascontiguousarray(inputs["w0_real"], dtype=np.float32)
    w0_imag = np.ascontiguousarray(inputs["w0_imag"], dtype=np.float32)
    wl_real = np.ascontiguousarray(inputs["wl_real"], dtype=np.float32)
    wl_imag = np.ascontiguousarray(inputs["wl_imag"], dtype=np.float32)

    xp_r = x_real[:, XPERM]
    xp_i = x_imag[:, XPERM]

    const_maps = {}
    for nm, arr in CONSTS.items():
        const_maps[nm] = np.ascontiguousarray(arr)
    in_maps = []
    for cid in range(NCORES):
        rows = slice(cid * BCORE, (cid + 1) * BCORE)
        m = {
            "xp_r": np.ascontiguousarray(xp_r[rows]),
            "xp_i": np.ascontiguousarray(xp_i[rows]),
            "w0r": w0_real, "w0i": w0_imag,
            "wlr": wl_real, "wli": wl_imag,
        }
        m.update(const_maps)
        in_maps.append(m)

    global _LAST_IN_MAPS
    _LAST_IN_MAPS = in_maps
    if _NC_CACHE is None:
        _NC_CACHE = build_nc()
    res = run_bass_kernel_spmd(_NC_CACHE, in_maps, core_ids=list(range(NCORES)))

    out = np.empty((B, CLASS_NUM), np.float32)
    cols = YN[YVALID] - CROP0
    for cid in range(NCORES):
        yraw = res.results[cid]["yraw"]  # [2176, 512]
        out[cid * BCORE : (cid + 1) * BCORE, cols] = yraw[YVALID].T
    return out
